# revision 6
# baseline (speedup 1.0000x reference)
"""BlockAttentionResidual Trainium2 kernel.

Math (per (b,t) row, V slice v_n of length D, n = 0..7):
    ssq_n = sum(v_n^2)
    rms_n = rsqrt(ssq_n / D + eps)
    logit_n = rms_n * dot(v_n, qw)        with qw = key_norm_weight * pseudo_query
    w = softmax(logit)                     over n
    out = sum_n w_n * v_n

Sharding: rows (B*T flattened) split evenly across 8 cores; (D,) params
replicated. No cross-core communication.

Per-core layout: tiles of 16 rows; SBUF tile (128, D) with partition
p = 8*r + n (r in 0..15, n in 0..7).
  - ssq: ScalarE activation(Square, accum_out)        one pass
  - dot: VectorE tensor_tensor_reduce(mult, add)      one pass
  - softmax over n: small PE transposes to put n on the free axis
  - weighted sum: PE matmul, stationary (128, 32) block-diagonal with
    w[r, n] at [8r+n, r]; 4 row-groups col-tiled into one (128, D) PSUM
    region, copied to SBUF by ScalarE, DMA'd to HBM.
"""

import os
import sys

for _p in ("/opt/trn_rl_repo",):
    if _p not in sys.path and os.path.isdir(_p):
        sys.path.append(_p)

import numpy as np

import concourse.bass as bass
import concourse.tile as tile
from concourse import bacc, mybir
from concourse.bass_utils import run_bass_kernel_spmd

N_CORES = 8
N = 8          # depth entries (softmax axis)
B = 4
T = 2048
D = 2048
R_TOTAL = B * T            # 8192 rows
RPC = R_TOTAL // N_CORES   # 1024 rows per core
TR = 16                    # rows per tile (16*8 = 128 partitions)
EPS = 1e-6
NCHUNK = 512               # matmul moving free-dim chunk (fp32 max)

F32 = mybir.dt.float32
ALU = mybir.AluOpType
ACTF = mybir.ActivationFunctionType


def build_program(rows_per_core=RPC, debug=False):
    """Build the per-core Bass program (identical on all cores)."""
    nt = rows_per_core // TR           # tiles per core
    nc = bacc.Bacc(
        "TRN2", target_bir_lowering=False, debug=debug, num_devices=N_CORES
    )

    v_dram = nc.dram_tensor("V", (N, rows_per_core, D), F32, kind="ExternalInput").ap()
    qw_dram = nc.dram_tensor("QW", (128, D), F32, kind="ExternalInput").ap()
    e32_dram = nc.dram_tensor("E32", (128, 32), F32, kind="ExternalInput").ap()
    id_dram = nc.dram_tensor("ID", (128, 128), F32, kind="ExternalInput").ap()
    out_dram = nc.dram_tensor(
        "OUT", (rows_per_core, D), F32, kind="ExternalOutput"
    ).ap()

    with tile.TileContext(nc) as tc:
        with (
            tc.tile_pool(name="consts", bufs=1) as consts,
            tc.tile_pool(name="xpool", bufs=12) as xpool,
            tc.tile_pool(name="scratch", bufs=2) as scratch,
            tc.tile_pool(name="outpool", bufs=2) as outpool,
            tc.tile_pool(name="smalls", bufs=3) as smalls,
            tc.tile_pool(name="wdpool", bufs=6) as wdpool,
            tc.tile_pool(name="psum_big", bufs=1, space="PSUM") as psum_big_pool,
            tc.tile_pool(name="psum_sm", bufs=2, space="PSUM") as psum_sm,
        ):
            qw_sb = consts.tile([128, D], F32)
            nc.sync.dma_start(qw_sb[:], qw_dram[:])
            e32_sb = consts.tile([128, 32], F32)
            nc.sync.dma_start(e32_sb[:], e32_dram[:])
            id_sb = consts.tile([128, 128], F32)
            nc.sync.dma_start(id_sb[:], id_dram[:])
            zero_sb = consts.tile([128, 1], F32)
            nc.vector.memset(zero_sb[:], 0.0)
            eps_sb = consts.tile([128, 1], F32)
            nc.vector.memset(eps_sb[:], EPS)

            assert nt % 8 == 0, "tiles per core must be a multiple of 8"
            for g in range(nt // 8):          # softmax groups of 8 tiles
                xt = []
                dots = smalls.tile([128, 8], F32, tag="dots")
                ssqs = smalls.tile([128, 8], F32, tag="ssqs")
                for j in range(8):
                    t = 8 * g + j
                    x = xpool.tile([128, D], F32, tag="x")
                    src = v_dram[:, TR * t : TR * (t + 1), :].transpose([1, 0, 2])
                    nc.sync.dma_start(x[:], src)
                    xt.append(x)

                    prod = scratch.tile([128, D], F32, tag="prod")
                    nc.vector.scalar_tensor_tensor(
                        out=prod[:],
                        in0=x[:],
                        scalar=1.0,
                        in1=qw_sb[:],
                        op0=ALU.mult,
                        op1=ALU.mult,
                        accum_out=dots[:, j : j + 1],
                    )
                    sq = scratch.tile([128, D], F32, tag="sq")
                    nc.scalar.activation(
                        sq[:], x[:], ACTF.Square, bias=zero_sb[:],
                        accum_out=ssqs[:, j : j + 1],
                    )

                # logits = dot * rsqrt(ssq/D + eps)  — (128, 8)
                snorm = smalls.tile([128, 8], F32, tag="snorm")
                nc.scalar.activation(
                    snorm[:], ssqs[:], ACTF.Sqrt, bias=eps_sb[:], scale=1.0 / D
                )
                rms = smalls.tile([128, 8], F32, tag="rms")
                nc.vector.reciprocal(rms[:], snorm[:])
                logits = smalls.tile([128, 8], F32, tag="logits")
                nc.vector.tensor_mul(logits[:], dots[:], rms[:])

                # transpose to (8, 128) so n is innermost on the free axis
                ps_t = psum_sm.tile([8, 128], F32, tag="pst")
                nc.tensor.transpose(ps_t[:], logits[:], id_sb[:])
                tsb = smalls.tile([8, 128], F32, tag="tsb")
                nc.scalar.copy(tsb[:], ps_t[:])
                t3 = tsb[:].rearrange("p (r n) -> p r n", n=N)

                negmax = smalls.tile([8, 16], F32, tag="negmax")
                nc.vector.tensor_reduce(
                    negmax[:], t3, axis=mybir.AxisListType.X, op=ALU.max, negate=True
                )
                shifted = smalls.tile([8, 128], F32, tag="shifted")
                sh3 = shifted[:].rearrange("p (r n) -> p r n", n=N)
                nmb = negmax[:].unsqueeze(2).broadcast_to([8, 16, N])
                nc.vector.tensor_tensor(sh3, t3, nmb, ALU.add)
                expd = smalls.tile([8, 128], F32, tag="expd")
                nc.scalar.activation(expd[:], shifted[:], ACTF.Exp, bias=zero_sb[0:8])
                ex3 = expd[:].rearrange("p (r n) -> p r n", n=N)
                sums = smalls.tile([8, 16], F32, tag="sums")
                nc.vector.tensor_reduce(
                    sums[:], ex3, axis=mybir.AxisListType.X, op=ALU.add
                )
                rsums = smalls.tile([8, 16], F32, tag="rsums")
                nc.vector.reciprocal(rsums[:], sums[:])
                wts = smalls.tile([8, 128], F32, tag="wts")
                w3 = wts[:].rearrange("p (r n) -> p r n", n=N)
                rsb = rsums[:].unsqueeze(2).broadcast_to([8, 16, N])
                nc.vector.tensor_tensor(w3, ex3, rsb, ALU.mult)

                # transpose back to one weight column per tile — (128, 8)
                ps_w = psum_sm.tile([128, 8], F32, tag="psw")
                nc.tensor.transpose(ps_w[:], wts[:], id_sb[0:8, 0:8])
                wcols = smalls.tile([128, 8], F32, tag="wcols")
                nc.scalar.copy(wcols[:], ps_w[:])

                # weighted sum via PE; 4 tiles col-tiled into one PSUM region
                for half in range(2):
                    psb = psum_big_pool.tile([128, D], F32, tag="psb")
                    for c in range(4):
                        j = 4 * half + c
                        t = 8 * g + j
                        wdiag = wdpool.tile([128, 32], F32, tag="wd")
                        nc.vector.tensor_scalar(
                            out=wdiag[:],
                            in0=e32_sb[:],
                            scalar1=wcols[:, j : j + 1],
                            scalar2=None,
                            op0=ALU.mult,
                        )
                        for k in range(D // NCHUNK):
                            nc.tensor.matmul(
                                psb[32 * c : 32 * (c + 1), NCHUNK * k : NCHUNK * (k + 1)],
                                wdiag[:],
                                xt[j][:, NCHUNK * k : NCHUNK * (k + 1)],
                                start=True,
                                stop=True,
                                tile_position=(0, 32 * c),
                            )
                    osb = outpool.tile([128, D], F32, tag="osb")
                    nc.scalar.copy(osb[:], psb[:])
                    t0 = 8 * g + 4 * half
                    for c in range(4):
                        r0 = TR * (t0 + c)
                        nc.sync.dma_start(
                            out_dram[r0 : r0 + TR, :],
                            osb[32 * c : 32 * c + TR, :],
                        )

    nc.compile()
    return nc


def make_consts():
    """Host-side constant tensors: E32 mask and 128x128 identity."""
    e32 = np.zeros((128, 32), dtype=np.float32)
    for p in range(128):
        e32[p, p // N] = 1.0
    ident = np.eye(128, dtype=np.float32)
    return e32, ident


def prepare_in_maps(V, key_norm_weight, pseudo_query, rows_per_core=RPC,
                    n_cores=N_CORES):
    qw = (np.asarray(key_norm_weight, dtype=np.float32)
          * np.asarray(pseudo_query, dtype=np.float32))
    qw_b = np.ascontiguousarray(np.broadcast_to(qw, (128, D)))
    e32, ident = make_consts()
    vf = np.ascontiguousarray(np.asarray(V, dtype=np.float32)).reshape(N, -1, D)
    in_maps = []
    for c in range(n_cores):
        sl = np.ascontiguousarray(
            vf[:, c * rows_per_core : (c + 1) * rows_per_core, :]
        )
        in_maps.append({"V": sl, "QW": qw_b, "E32": e32, "ID": ident})
    return in_maps


_PROGRAM_CACHE = {}


def _get_program():
    key = (RPC,)
    if key not in _PROGRAM_CACHE:
        _PROGRAM_CACHE[key] = build_program(RPC, debug=False)
    return _PROGRAM_CACHE[key]


def run(V, key_norm_weight, pseudo_query, trace=False, **trace_kwargs):
    nc = _get_program()
    in_maps = prepare_in_maps(V, key_norm_weight, pseudo_query)
    res = run_bass_kernel_spmd(
        nc, in_maps, list(range(N_CORES)), trace=trace, **trace_kwargs
    )
    out = np.empty((R_TOTAL, D), dtype=np.float32)
    for c in range(N_CORES):
        out[c * RPC : (c + 1) * RPC, :] = res.results[c]["OUT"]
    return out.reshape(B, T, D), res


def kernel(V, key_norm_weight, pseudo_query):
    out, _ = run(V, key_norm_weight, pseudo_query, trace=False)
    return out
